# revision 1
# baseline (speedup 1.0000x reference)
"""MultiHeadAttention (B=4, S=2048, D=1024, H=16, causal + key mask) on 8 trn2 cores.

Sharding: Megatron-style tensor parallel over heads. Each core owns 2 heads:
column slices of Wq/Wk/Wv (D x 128), the matching row slice of Wp (128 x D).
Each core computes a partial output y_c = attn_c @ Wp_c; host sums the 8
partials and adds bp.

Per-core kernel (all matmuls float32r: full PE rate at N=512, ~2e-4 rel err):
  - x^T [D, B*S] streamed in chunks; projections produce Q^T/K^T
    [128 = 2 heads x 64, B, S] directly (W slice as lhsT, x^T as rhs).
  - V via PE transpose into [s, hd] layout + a ones column so the PV matmul
    also accumulates the softmax denominator (row 64 of the PV psum).
  - Scores computed transposed: S^T[k, q] = K^T_slice.T @ Q^T_slice (K=64).
    Both heads' score blocks land in one [128,1024] 2-bank PSUM tile ->
    ONE additive causal mask (DVE) + ONE exp (ScalarE, key-mask as
    per-partition bias) per k-block. No max-subtraction (logits are O(1)).
  - PV accumulates attnT[hd, q]; psum copied to SBUF fast (frees the bank),
    reciprocal (DVE) -> partition-broadcast (DMA) -> normalize into a
    dedicated attnT buffer (reuses the x-stream pool's SBUF space).
  - Output projection is emitted two groups behind so the normalize chain
    never head-of-line blocks the in-order PE queue.
"""

import numpy as np

P = 128
B, S, D, H = 4, 2048, 1024, 16
HD = D // H  # 64
NCORES = 8
HPC = H // NCORES  # 2 heads per core
BS = B * S  # 8192
NB = S // P  # 16 k-blocks per batch
NG = S // 512  # 4 q-groups per batch

_CACHE = {}


def _build_nc():
    import concourse.mybir as mybir
    from concourse import bacc
    from concourse.tile import TileContext
    from concourse.masks import make_identity
    from contextlib import ExitStack

    f32 = mybir.dt.float32
    f32r = mybir.dt.float32r
    AF = mybir.ActivationFunctionType

    nc = bacc.Bacc("TRN2", target_bir_lowering=False, debug=False,
                   num_devices=NCORES)

    xT_d = nc.dram_tensor("xT", [D, BS], f32r, kind="ExternalInput").ap()
    wq_d = nc.dram_tensor("wq", [D, P], f32r, kind="ExternalInput").ap()
    wk_d = nc.dram_tensor("wk", [D, P], f32r, kind="ExternalInput").ap()
    wv_d = nc.dram_tensor("wv", [D, P], f32r, kind="ExternalInput").ap()
    bq_d = nc.dram_tensor("bq", [P, 1], f32, kind="ExternalInput").ap()
    bk_d = nc.dram_tensor("bk", [P, 1], f32, kind="ExternalInput").ap()
    bv_d = nc.dram_tensor("bv", [P, 1], f32, kind="ExternalInput").ap()
    wp_d = nc.dram_tensor("wp", [P, D], f32r, kind="ExternalInput").ap()
    mb_d = nc.dram_tensor("maskb", [P, B * NB], f32, kind="ExternalInput").ap()
    cm_d = nc.dram_tensor("cmask", [P, 4, 1024], f32,
                          kind="ExternalInput").ap()
    yp_d = nc.dram_tensor("yp", [BS, D], f32, kind="ExternalOutput").ap()

    xT_r = xT_d.rearrange("(o p) n -> p o n", p=P)  # [128, 8, 8192]
    KD = D // P  # 8 contraction chunks

    with TileContext(nc) as tc:
        with ExitStack() as ctx:
            consts = ctx.enter_context(tc.tile_pool(name="consts", bufs=1))
            big = ctx.enter_context(tc.tile_pool(name="big", bufs=1))
            ptpool = ctx.enter_context(tc.tile_pool(name="ptpool", bufs=3))
            npool = ctx.enter_context(tc.tile_pool(name="npool", bufs=2))
            ypool = ctx.enter_context(tc.tile_pool(name="ypool", bufs=3))
            psum = ctx.enter_context(
                tc.tile_pool(name="psum", bufs=2, space="PSUM"))
            sc2pool = ctx.enter_context(
                tc.tile_pool(name="sc2pool", bufs=2, space="PSUM"))
            pvpool = ctx.enter_context(
                tc.tile_pool(name="pvpool", bufs=2, space="PSUM"))

            # ---- constants ----
            wq_sb = consts.tile([P, KD, P], f32r, tag="wq")
            wk_sb = consts.tile([P, KD, P], f32r, tag="wk")
            wv_sb = consts.tile([P, KD, P], f32r, tag="wv")
            nc.sync.dma_start(wq_sb[:], wq_d.rearrange("(o p) m -> p o m", p=P))
            nc.sync.dma_start(wk_sb[:], wk_d.rearrange("(o p) m -> p o m", p=P))
            nc.sync.dma_start(wv_sb[:], wv_d.rearrange("(o p) m -> p o m", p=P))
            wp_sb = consts.tile([P, D], f32r, tag="wp")
            nc.sync.dma_start(wp_sb[:], wp_d)
            bq_sb = consts.tile([P, 1], f32, tag="bq")
            bk_sb = consts.tile([P, 1], f32, tag="bk")
            bv_sb = consts.tile([P, 1], f32, tag="bv")
            nc.sync.dma_start(bq_sb[:], bq_d)
            nc.sync.dma_start(bk_sb[:], bk_d)
            nc.sync.dma_start(bv_sb[:], bv_d)
            mb_sb = consts.tile([P, B * NB], f32, tag="mb")
            nc.sync.dma_start(mb_sb[:], mb_d)
            cm_sb = consts.tile([P, 4, 1024], f32, tag="cm")
            nc.sync.dma_start(cm_sb[:], cm_d)
            ident = consts.tile([P, P], f32, tag="ident")
            make_identity(nc, ident[:])

            # ---- persistent activations ----
            qt_sb = big.tile([P, B, S], f32r, tag="qt")  # Q^T
            kt_sb = big.tile([P, B, S], f32r, tag="kt")  # K^T
            # V in [s, hd] layout + ones col: [p=s%128, h, b, sblock, 65]
            v_sb = big.tile([P, HPC, B, NB, HD + 1], f32r, tag="v")
            nc.vector.memset(v_sb[:, :, :, :, HD].bitcast(f32), 1.0)

            # ---- phase 1: projections (x-stream pools scoped here) ----
            with tc.tile_pool(name="xpool", bufs=2) as xpool, \
                 tc.tile_pool(name="vtpool", bufs=2) as vtpool:
                for c in range(BS // 512):  # 16 chunks of 512 rows, b-major
                    b, sc = divmod(c, NG)
                    xt = xpool.tile([P, KD, 512], f32r, tag="xt")
                    nc.sync.dma_start(xt[:], xT_r[:, :, c * 512:(c + 1) * 512])
                    ssl = slice(sc * 512, (sc + 1) * 512)

                    for which in range(3):
                        w_sb = (wq_sb, wk_sb, wv_sb)[which]
                        ps = psum.tile([P, 512], f32, tag="ps")
                        for o in range(KD):
                            nc.tensor.matmul(
                                ps[:], lhsT=w_sb[:, o, :], rhs=xt[:, o, :],
                                start=(o == 0), stop=(o == KD - 1))
                        if which == 0:
                            nc.scalar.activation(qt_sb[:, b, ssl], ps[:],
                                                 AF.Identity, bias=bq_sb[:])
                        elif which == 1:
                            nc.scalar.activation(kt_sb[:, b, ssl], ps[:],
                                                 AF.Identity, bias=bk_sb[:])
                        else:
                            vt = vtpool.tile([P, 512], f32, tag="vt")
                            nc.scalar.activation(vt[:], ps[:], AF.Identity,
                                                 bias=bv_sb[:])
                            for t in range(4):
                                trp = psum.tile([P, 512], f32, tag="ps")
                                nc.tensor.transpose(
                                    trp[:, :P], vt[:, t * P:(t + 1) * P],
                                    ident[:])
                                sb_i = sc * 4 + t
                                nc.vector.tensor_copy(
                                    v_sb[:, 0, b, sb_i, 0:HD], trp[:, 0:HD])
                                nc.vector.tensor_copy(
                                    v_sb[:, 1, b, sb_i, 0:HD],
                                    trp[:, HD:2 * HD])

            # attnT buffer (reuses the closed x-stream pools' SBUF space)
            atpool = ctx.enter_context(tc.tile_pool(name="atpool", bufs=1))
            at_sb = atpool.tile([P, B, S], f32r, tag="at")

            # ---- phase 2: attention + output projection ----
            def outproj(b, g):
                for qc in range(4):
                    q0 = g * 512 + qc * P
                    r0 = b * S + q0
                    y_sb = ypool.tile([P, D], f32, tag="y",
                                      name=f"y_{b}_{g}_{qc}")
                    for half in range(2):
                        yp_ps = psum.tile([P, 512], f32, tag="ps",
                                          name=f"yps_{b}_{g}_{qc}_{half}")
                        nc.tensor.matmul(
                            yp_ps[:],
                            lhsT=at_sb[:, b, q0:q0 + P],
                            rhs=wp_sb[:, half * 512:(half + 1) * 512],
                            start=True, stop=True)
                        ysl = y_sb[:, half * 512:(half + 1) * 512]
                        nc.scalar.activation(ysl, yp_ps[:], AF.Copy)
                    nc.sync.dma_start(yp_d[r0:r0 + P, :], y_sb[:])

            pending = []
            for b in range(B):
                for g in range(NG):
                    gsl = slice(g * 512, (g + 1) * 512)
                    nkb = 4 * (g + 1)
                    pvs = [pvpool.tile([P, 512], f32, tag="pv",
                                       name=f"pv_{b}_{g}_{h}")
                           for h in range(HPC)]
                    for kb in range(nkb):
                        j = kb - 4 * g
                        col = b * NB + kb
                        # deep-diagonal blocks (j>=2): q < 128*j is fully
                        # masked; restrict to q in [256,512) (N=256 keeps
                        # full f32r rate; contiguous APs only)
                        qo = 256 if j >= 2 else 0
                        sc2 = sc2pool.tile([P, 1024], f32, tag="sc2",
                                           name=f"sc2_{b}_{g}_{kb}")
                        for h in range(HPC):
                            hsl = slice(h * HD, (h + 1) * HD)
                            nc.tensor.matmul(
                                sc2[:, h * 512 + qo:(h + 1) * 512],
                                lhsT=kt_sb[hsl, b, kb * P:(kb + 1) * P],
                                rhs=qt_sb[hsl, b,
                                          g * 512 + qo:(g + 1) * 512],
                                start=True, stop=True)
                        pt = ptpool.tile([P, 1024], f32r, tag="pt")
                        if qo == 0:
                            if j >= 0:  # diagonal: additive causal mask
                                nc.vector.tensor_add(sc2[:], sc2[:],
                                                     cm_sb[:, j, :])
                            nc.scalar.activation(pt[:], sc2[:], AF.Exp,
                                                 bias=mb_sb[:, col:col + 1])
                        else:
                            for h in range(HPC):
                                hs = slice(h * 512 + qo, (h + 1) * 512)
                                nc.vector.tensor_add(sc2[:, hs], sc2[:, hs],
                                                     cm_sb[:, j, hs])
                                nc.scalar.activation(
                                    pt[:, hs], sc2[:, hs], AF.Exp,
                                    bias=mb_sb[:, col:col + 1])
                        for h in range(HPC):
                            nc.tensor.matmul(
                                pvs[h][0:HD + 1, qo:512],
                                lhsT=v_sb[:, h, b, kb, :],
                                rhs=pt[:, h * 512 + qo:(h + 1) * 512],
                                start=(kb == 0), stop=(kb == nkb - 1))
                    if len(pending) >= 2:
                        outproj(*pending.pop(0))
                    pending.append((b, g))
                    for h in range(HPC):
                        # free the pv psum slot fast: copy [65,512] to SBUF
                        pvs_sb = npool.tile([P, 512], f32, tag="pvs")
                        nc.scalar.activation(pvs_sb[0:HD + 1, :],
                                             pvs[h][0:HD + 1, :], AF.Copy)
                        # 1/sum(exp) (row 64), broadcast to 64 partitions
                        rec = npool.tile([P, 512], f32, tag="rec")
                        nc.vector.reciprocal(
                            rec[HD:HD + 1, :], pvs_sb[HD:HD + 1, :])
                        sx = npool.tile([HD, 512], f32, tag="sx")
                        nc.sync.dma_start(
                            sx[:],
                            rec[HD:HD + 1, None, :]
                            .to_broadcast((1, HD, 512)))
                        if h == 0:
                            nc.vector.tensor_mul(
                                at_sb[0:HD, b, gsl], pvs_sb[0:HD, :], sx[:])
                        else:
                            tmp = npool.tile([HD, 512], f32r, tag="tmp")
                            nc.vector.tensor_mul(
                                tmp[:], pvs_sb[0:HD, :], sx[:])
                            nc.sync.dma_start(at_sb[HD:2 * HD, b, gsl],
                                              tmp[:])

            for pg in pending:
                outproj(*pg)

    nc.compile()
    return nc


def _get_nc():
    if "nc" not in _CACHE:
        _CACHE["nc"] = _build_nc()
    return _CACHE["nc"]


def make_in_maps(x, attention_mask, Wq, bq, Wk, bk, Wv, bv, Wp, bp):
    """Host-side sharding: build the 8 per-core device input maps."""
    x = np.asarray(x, dtype=np.float32)
    scale = np.float32(1.0 / np.sqrt(HD))
    xT = np.ascontiguousarray(x.reshape(BS, D).T)  # [D, BS]
    mb = (np.asarray(attention_mask).astype(np.float32) - 1.0) * np.float32(1e9)
    mb = np.ascontiguousarray(
        mb.reshape(B, NB, P).transpose(2, 0, 1).reshape(P, B * NB))
    # causal diag masks (additive): 0 where 128*j + p <= f, else -1e9;
    # duplicated for the two head halves of the [128,1024] scores tile.
    pp = np.arange(P)[:, None]
    ff = np.arange(512)[None, :]
    cm = np.stack(
        [np.where(P * j + pp <= ff, 0.0, -1e9).astype(np.float32)
         for j in range(4)], axis=1)  # [128, 4, 512]
    cm = np.ascontiguousarray(np.concatenate([cm, cm], axis=-1))

    Wq = np.asarray(Wq, np.float32) * scale
    bq = np.asarray(bq, np.float32) * scale
    Wk = np.asarray(Wk, np.float32)
    bk = np.asarray(bk, np.float32)
    Wv = np.asarray(Wv, np.float32)
    bv = np.asarray(bv, np.float32)
    Wp = np.asarray(Wp, np.float32)

    in_maps = []
    for c in range(NCORES):
        cs = slice(c * P, (c + 1) * P)
        in_maps.append({
            "xT": xT,
            "wq": np.ascontiguousarray(Wq[:, cs]),
            "wk": np.ascontiguousarray(Wk[:, cs]),
            "wv": np.ascontiguousarray(Wv[:, cs]),
            "bq": np.ascontiguousarray(bq[cs].reshape(P, 1)),
            "bk": np.ascontiguousarray(bk[cs].reshape(P, 1)),
            "bv": np.ascontiguousarray(bv[cs].reshape(P, 1)),
            "wp": np.ascontiguousarray(Wp[cs, :]),
            "maskb": mb,
            "cmask": cm,
        })
    return in_maps


def run(inputs, trace=False, tmpdir=None):
    """Compile (cached) + run on 8 cores. Returns (output, BassKernelResults)."""
    from concourse import bass_utils
    nc = _get_nc()
    in_maps = make_in_maps(**inputs)
    kwargs = {}
    if trace:
        kwargs = dict(trace=True, tmpdir=tmpdir)
    res = bass_utils.run_bass_kernel_spmd(
        nc, in_maps, core_ids=list(range(NCORES)), **kwargs)
    acc = np.zeros((BS, D), dtype=np.float64)
    for r in res.results:
        acc += r["yp"].astype(np.float64)
    out = (acc + np.asarray(inputs["bp"], np.float64)[None, :]).astype(
        np.float32)
    return out.reshape(B, S, D), res


def kernel(**inputs) -> np.ndarray:
    out, _ = run(inputs, trace=False)
    return out



# revision 21
# speedup vs baseline: 1.2169x; 1.2169x over previous
"""MultiHeadAttention (B=4, S=2048, D=1024, H=16, causal + key mask) on 8 trn2 cores.

Sharding: batch x head-group. Core (b, hg) owns batch b and 8 heads (4 pairs
of 2). Host sums the two half-partials per batch and adds bp + bv@Wp (the V
bias is equivalent to a constant output offset because softmax rows sum to 1).

Per-core kernel (bf16 data path, fp32 PSUM/softmax stats):
  - x^T for its batch streamed in 4 chunks of 512 rows; projections produce
    Q^T/K^T [feat=128 (pair-of-heads), rows] per pair (W slice as lhsT).
    Q/K biases folded into the PSUM->SBUF evacuation (ScalarE Identity+bias).
  - V transposed into [row, hd] layout via PE transpose; a ones column per
    head (cols 64/130 of a 132-wide slot) makes the PV matmul accumulate the
    softmax denominator in psum row 64.
  - Scores computed transposed: S^T[k, q] = K^T.T @ Q^T (K=64 -> the two
    heads' matmuls auto-pack as 64x128 row tiles and run concurrently).
  - Causal masking only touches the [128, 2, 128] diagonal square of each
    diagonal block (the triangle pattern is j-invariant); the strictly-lower
    rectangle is exp'd unmasked. Off-diagonal q-columns are never computed.
  - exp on ScalarE (key mask as per-partition bias), output bf16.
  - Softmax denominators inverted with reciprocal_approx_fast (DVE custom
    op), broadcast via DMA, normalization multiply on DVE -> attnT bf16.
  - Output projection accumulates the 4 pairs in PSUM; evacuation on DVE in
    bf16; partial outputs DMA'd to HBM in bf16.
  - Projection chunk c feeds attention group g=c (causality), so projection
    PE work overlaps the ScalarE-bound attention of earlier groups.
"""

import os

import numpy as np

DBG = bool(int(os.environ.get("MHADBG", "0")))

P = 128
B, S, D, H = 4, 2048, 1024, 16
HD = D // H          # 64
NCORES = 8
NP = 4               # head pairs per core (8 heads)
NC = S // 512        # 4 row chunks
NB = S // P          # 16 key blocks
KD = D // P          # 8 contraction chunks
FPC = NP * P         # 512 features per core
VW = 2 * (HD + 2)    # 132: per-pair V slot width (64 + ones + pad, x2 heads)

_CACHE = {}


def _build_nc():
    import concourse.mybir as mybir
    from concourse import bacc
    from concourse.tile import TileContext
    from concourse.masks import make_identity
    from contextlib import ExitStack

    f32 = mybir.dt.float32
    bf16 = mybir.dt.bfloat16
    AF = mybir.ActivationFunctionType

    nc = bacc.Bacc("TRN2", target_bir_lowering=False, debug=False,
                   num_devices=NCORES)

    xT_d = nc.dram_tensor("xT", [P, NC, KD, 512], bf16,
                          kind="ExternalInput").ap()
    wq_d = nc.dram_tensor("wq", [P, KD, FPC], bf16, kind="ExternalInput").ap()
    wk_d = nc.dram_tensor("wk", [P, KD, FPC], bf16, kind="ExternalInput").ap()
    wv_d = nc.dram_tensor("wv", [P, KD, FPC], bf16, kind="ExternalInput").ap()
    bq_d = nc.dram_tensor("bq", [P, NP], f32, kind="ExternalInput").ap()
    bk_d = nc.dram_tensor("bk", [P, NP], f32, kind="ExternalInput").ap()
    wp_d = nc.dram_tensor("wp", [P, NP, D], bf16, kind="ExternalInput").ap()
    mb_d = nc.dram_tensor("maskb", [P, NB], f32, kind="ExternalInput").ap()
    cm_d = nc.dram_tensor("cmask", [P, 2, P], f32, kind="ExternalInput").ap()
    yp_d = nc.dram_tensor("yp", [S, D], bf16, kind="ExternalOutput").ap()
    if DBG:
        dbg_qt = nc.dram_tensor("dbg_qt", [P, NC, NP, 512], bf16,
                                kind="ExternalOutput").ap()
        dbg_kt = nc.dram_tensor("dbg_kt", [P, NC, NP, 512], bf16,
                                kind="ExternalOutput").ap()
        dbg_v = nc.dram_tensor("dbg_v", [P, NC, NP, 4, VW], bf16,
                               kind="ExternalOutput").ap()
        dbg_at = nc.dram_tensor("dbg_at", [P, NC, NP, 512], bf16,
                                kind="ExternalOutput").ap()

    with TileContext(nc) as tc:
        with ExitStack() as ctx:
            consts = ctx.enter_context(tc.tile_pool(name="consts", bufs=1))
            big = ctx.enter_context(tc.tile_pool(name="big", bufs=1))
            xpool = ctx.enter_context(tc.tile_pool(name="xpool", bufs=2))
            vtpool = ctx.enter_context(tc.tile_pool(name="vtpool", bufs=2))
            ptpool = ctx.enter_context(tc.tile_pool(name="ptpool", bufs=3))
            npool = ctx.enter_context(tc.tile_pool(name="npool", bufs=2))
            pvsbpool = ctx.enter_context(
                tc.tile_pool(name="pvsbpool", bufs=9))
            ypool = ctx.enter_context(tc.tile_pool(name="ypool", bufs=3))
            pspool = ctx.enter_context(
                tc.tile_pool(name="pspool", bufs=2, space="PSUM"))
            sc2pool = ctx.enter_context(
                tc.tile_pool(name="sc2pool", bufs=2, space="PSUM"))
            pvpool = ctx.enter_context(
                tc.tile_pool(name="pvpool", bufs=2, space="PSUM"))

            # ---- constants ----
            wq_sb = consts.tile([P, KD, FPC], bf16, tag="wq")
            wk_sb = consts.tile([P, KD, FPC], bf16, tag="wk")
            wv_sb = consts.tile([P, KD, FPC], bf16, tag="wv")
            wp_sb = consts.tile([P, NP, D], bf16, tag="wp")
            nc.sync.dma_start(wq_sb[:], wq_d)
            nc.sync.dma_start(wk_sb[:], wk_d)
            nc.sync.dma_start(wv_sb[:], wv_d)
            nc.sync.dma_start(wp_sb[:], wp_d)
            bq_sb = consts.tile([P, NP], f32, tag="bq")
            bk_sb = consts.tile([P, NP], f32, tag="bk")
            nc.sync.dma_start(bq_sb[:], bq_d)
            nc.sync.dma_start(bk_sb[:], bk_d)
            mb_sb = consts.tile([P, NB], f32, tag="mb")
            nc.sync.dma_start(mb_sb[:], mb_d)
            cm_sb = consts.tile([P, 2, P], f32, tag="cm")
            nc.sync.dma_start(cm_sb[:], cm_d)
            ident = consts.tile([P, P], f32, tag="ident")
            make_identity(nc, ident[:])

            # ---- per-chunk activations (distinct tiles -> clean deps) ----
            qt_c = [big.tile([P, NP, 512], bf16, tag=f"qt{c}",
                             name=f"qt{c}") for c in range(NC)]
            kt_c = [big.tile([P, NP, 512], bf16, tag=f"kt{c}",
                             name=f"kt{c}") for c in range(NC)]
            v_c = [big.tile([P, NP, 4, VW], bf16, tag=f"v{c}",
                            name=f"v{c}") for c in range(NC)]
            at_gp = [[big.tile([P, 512], bf16, tag=f"at{g}_{p}",
                               name=f"at{g}_{p}") for p in range(NP)]
                     for g in range(NC)]
            for c in range(NC):
                nc.vector.memset(v_c[c][:, :, :, HD], 1.0)
                nc.vector.memset(v_c[c][:, :, :, HD + 2 + HD], 1.0)

            def proj_part(c, which, xt):
                w_sb = (wq_sb, wk_sb, wv_sb)[which]
                for mt in range(NP):
                    if True:
                        ps = pspool.tile([P, 512], f32, tag="ps",
                                         name=f"pj_{c}_{which}_{mt}")
                        for o in range(KD):
                            nc.tensor.matmul(
                                ps[:], lhsT=w_sb[:, o, mt * P:(mt + 1) * P],
                                rhs=xt[:, o, :],
                                start=(o == 0), stop=(o == KD - 1))
                        if which == 0:
                            nc.scalar.activation(qt_c[c][:, mt, :], ps[:],
                                                 AF.Identity,
                                                 bias=bq_sb[:, mt:mt + 1])
                        elif which == 1:
                            nc.scalar.activation(kt_c[c][:, mt, :], ps[:],
                                                 AF.Identity,
                                                 bias=bk_sb[:, mt:mt + 1])
                        else:
                            vt = vtpool.tile([P, 512], f32, tag="vt")
                            nc.vector.tensor_copy(vt[:], ps[:])
                            for t in range(4):
                                trp = pspool.tile([P, P], f32, tag="ps",
                                                  name=f"tr_{c}_{mt}_{t}")
                                nc.tensor.transpose(
                                    trp[:], vt[:, t * P:(t + 1) * P],
                                    ident[:])
                                dst = (v_c[c][:, mt, t, :]
                                       .rearrange("p (h x) -> p h x", h=2)
                                       [:, :, 0:HD])
                                src = trp[:].rearrange(
                                    "p (h x) -> p h x", h=2)
                                nc.vector.tensor_copy(dst, src)

            def attention(pair, g, pvsb_g, dn_g):
                nkb = 4 * (g + 1)
                pvs = [pvpool.tile([P, 512], f32, tag="pv",
                                   name=f"pv_{pair}_{g}_{h}")
                       for h in range(2)]
                for kb in range(nkb):
                    j = kb - 4 * g
                    q0 = P * j if j >= 0 else 0
                    sc2 = sc2pool.tile([P, 1024], f32, tag="sc2",
                                       name=f"sc2_{pair}_{g}_{kb}")
                    for h in range(2):
                        hsl = slice(HD * h, HD * (h + 1))
                        nc.tensor.matmul(
                            sc2[:, h * 512 + q0:(h + 1) * 512],
                            lhsT=kt_c[kb // 4][hsl, pair,
                                               (kb % 4) * P:(kb % 4 + 1) * P],
                            rhs=qt_c[g][hsl, pair, q0:512],
                            start=True, stop=True)
                    sc2r = sc2[:].rearrange("p (h q) -> p h q", h=2)
                    pt = ptpool.tile([P, 2, 512], bf16, tag="pt")
                    mbc = mb_sb[:, kb:kb + 1]
                    if j >= 0:
                        sq = sc2r[:, :, q0:q0 + P]
                        nc.vector.tensor_add(sq, sq, cm_sb[:])
                        nc.scalar.activation(pt[:, :, q0:512],
                                             sc2r[:, :, q0:512],
                                             AF.Exp, bias=mbc)
                    else:
                        nc.scalar.activation(pt[:], sc2r, AF.Exp, bias=mbc)
                    for h in range(2):
                        vsl = (v_c[kb // 4][:, pair, kb % 4, :]
                               .rearrange("p (hh x) -> p hh x", hh=2)
                               [:, h, 0:HD + 1])
                        nc.tensor.matmul(
                            pvs[h][0:HD + 1, q0:512],
                            lhsT=vsl,
                            rhs=pt[:, h, q0:512],
                            start=(kb == 0), stop=(kb == nkb - 1))
                for h in range(2):
                    r = 2 * pair + h
                    pvsb = pvsbpool.tile([P, 512], f32, tag="pvsb",
                                         name=f"pvsb_{pair}_{g}_{h}")
                    nc.vector.tensor_copy(pvsb[0:HD + 1, :],
                                          pvs[h][0:HD + 1, :])
                    # gather the denominator row into dn_g (partition move)
                    nc.sync.dma_start(dn_g[r:r + 1, :], pvsb[HD:HD + 1, :])
                    pvsb_g[r] = pvsb

            def normalize(g, pvsb_g, dn_g):
                rcg = npool.tile([8, 512], f32, tag="rcg")
                nc.vector.reciprocal(rcg[:], dn_g[:])
                for pair in range(NP):
                    for h in range(2):
                        sx = npool.tile([HD, 512], f32, tag="sx")
                        r = 2 * pair + h
                        nc.sync.dma_start(
                            sx[:],
                            rcg[r:r + 1, None, :].to_broadcast((1, HD, 512)))
                        if h == 0:
                            nc.vector.tensor_mul(
                                at_gp[g][pair][0:HD, :],
                                pvsb_g[r][0:HD, :], sx[:])
                        else:
                            tmp = npool.tile([HD, 512], bf16, tag="tmp")
                            nc.vector.tensor_mul(
                                tmp[:], pvsb_g[r][0:HD, :], sx[:])
                            nc.sync.dma_start(
                                at_gp[g][pair][HD:2 * HD, :], tmp[:])

            def outproj(g):
                for qi in range(4):
                    q0 = g * 512 + qi * P
                    yb = ypool.tile([P, D], bf16, tag="yb")
                    for half in range(2):
                        ps = pspool.tile([P, 512], f32, tag="ps",
                                         name=f"yps_{g}_{qi}_{half}")
                        for pair in range(NP):
                            nc.tensor.matmul(
                                ps[:],
                                lhsT=at_gp[g][pair][:, qi * P:(qi + 1) * P],
                                rhs=wp_sb[:, pair,
                                          half * 512:(half + 1) * 512],
                                start=(pair == 0), stop=(pair == NP - 1))
                        nc.vector.tensor_copy(
                            yb[:, half * 512:(half + 1) * 512], ps[:])
                    nc.sync.dma_start(yp_d[q0:q0 + P, :], yb[:])

            # chunk 0 projection (head of the pipeline)
            xt0 = xpool.tile([P, KD, 512], bf16, tag="xt", name="xt0")
            nc.sync.dma_start(xt0[:], xT_d[:, 0])
            for w in range(3):
                proj_part(0, w, xt0)
            # steady state: attention group g = c-1 overlaps projection of
            # chunk c (group g only needs chunks <= g)
            for c in range(1, NC + 1):
                g = c - 1
                if c < NC:
                    xt = xpool.tile([P, KD, 512], bf16, tag="xt",
                                    name=f"xt{c}")
                    nc.sync.dma_start(xt[:], xT_d[:, c])
                pvsb_g = {}
                dn_g = npool.tile([8, 512], f32, tag="dn", name=f"dn{g}")
                for pair in range(NP):
                    if c < NC and pair < 3:
                        proj_part(c, pair, xt)
                    attention(pair, g, pvsb_g, dn_g)
                    if pair == 0 and g >= 1:
                        outproj(g - 1)
                normalize(g, pvsb_g, dn_g)
            outproj(NC - 1)

            if DBG:
                for c in range(NC):
                    nc.sync.dma_start(dbg_qt[:, c], qt_c[c][:])
                    nc.sync.dma_start(dbg_kt[:, c], kt_c[c][:])
                    nc.sync.dma_start(dbg_v[:, c], v_c[c][:])
                    nc.sync.dma_start(dbg_at[:, c], at_g[c][:])

    nc.compile()
    return nc


def _get_nc():
    if "nc" not in _CACHE:
        _CACHE["nc"] = _build_nc()
    return _CACHE["nc"]


def make_in_maps(x, attention_mask, Wq, bq, Wk, bk, Wv, bv, Wp, bp):
    """Host-side sharding: 8 per-core input maps (core = b * 2 + hg)."""
    import ml_dtypes
    bf = ml_dtypes.bfloat16
    x = np.asarray(x, dtype=np.float32)
    scale = np.float32(1.0 / np.sqrt(HD))
    Wq = np.asarray(Wq, np.float32) * scale
    bq = np.asarray(bq, np.float32) * scale
    Wk = np.asarray(Wk, np.float32)
    bk = np.asarray(bk, np.float32)
    Wv = np.asarray(Wv, np.float32)
    Wp = np.asarray(Wp, np.float32)
    mask = np.asarray(attention_mask).astype(np.float32)

    # causal triangle for the diagonal 128x128 square (j-invariant):
    # row p (key), col q: valid iff p <= q.
    pp = np.arange(P)[:, None]
    qq = np.arange(P)[None, :]
    tri = np.where(pp <= qq, 0.0, -1e9).astype(np.float32)
    cm = np.ascontiguousarray(
        np.broadcast_to(tri[:, None, :], (P, 2, P)))

    in_maps = []
    for core in range(NCORES):
        b, hg = divmod(core, 2)
        fsl = slice(hg * FPC, (hg + 1) * FPC)
        xT = x[b].T.reshape(KD, P, NC, 512).transpose(1, 2, 0, 3)
        mb = ((mask[b] - 1.0) * np.float32(1e9)).reshape(NB, P).T
        in_maps.append({
            "xT": np.ascontiguousarray(xT.astype(bf)),
            "wq": np.ascontiguousarray(
                Wq[:, fsl].reshape(KD, P, FPC).transpose(1, 0, 2).astype(bf)),
            "wk": np.ascontiguousarray(
                Wk[:, fsl].reshape(KD, P, FPC).transpose(1, 0, 2).astype(bf)),
            "wv": np.ascontiguousarray(
                Wv[:, fsl].reshape(KD, P, FPC).transpose(1, 0, 2).astype(bf)),
            "bq": np.ascontiguousarray(bq[fsl].reshape(NP, P).T),
            "bk": np.ascontiguousarray(bk[fsl].reshape(NP, P).T),
            "wp": np.ascontiguousarray(
                Wp[fsl, :].reshape(NP, P, D).transpose(1, 0, 2).astype(bf)),
            "maskb": np.ascontiguousarray(mb),
            "cmask": cm,
        })
    return in_maps


def run(inputs, trace=False, tmpdir=None):
    """Compile (cached) + run on 8 cores. Returns (output, BassKernelResults)."""
    from concourse import bass_utils
    nc = _get_nc()
    in_maps = make_in_maps(**inputs)
    kwargs = {}
    if trace:
        kwargs = dict(trace=True, tmpdir=tmpdir)
    res = bass_utils.run_bass_kernel_spmd(
        nc, in_maps, core_ids=list(range(NCORES)), **kwargs)
    bv = np.asarray(inputs["bv"], np.float64)
    Wp = np.asarray(inputs["Wp"], np.float64)
    bp = np.asarray(inputs["bp"], np.float64)
    const = bv @ Wp + bp  # V-bias contribution (softmax rows sum to 1) + bp
    out = np.empty((B, S, D), dtype=np.float32)
    for b in range(B):
        acc = (res.results[2 * b]["yp"].astype(np.float64)
               + res.results[2 * b + 1]["yp"].astype(np.float64) + const)
        out[b] = acc.astype(np.float32)
    return out, res


def kernel(**inputs) -> np.ndarray:
    out, _ = run(inputs, trace=False)
    return out


# revision 23
# speedup vs baseline: 1.2235x; 1.0054x over previous
"""MultiHeadAttention (B=4, S=2048, D=1024, H=16, causal + key mask) on 8 trn2 cores.

Sharding: batch x head-group. Core (b, hg) owns batch b and 8 heads (4 pairs
of 2). Host sums the two half-partials per batch and adds bp + bv@Wp (the V
bias is equivalent to a constant output offset because softmax rows sum to 1).

Per-core kernel (bf16 data path, fp32 PSUM/softmax stats):
  - x^T for its batch streamed in 4 chunks of 512 rows; projections produce
    Q^T/K^T [feat=128 (pair-of-heads), rows] per pair (W slice as lhsT).
    Q/K biases folded into the PSUM->SBUF evacuation (ScalarE Identity+bias).
  - V transposed into [row, hd] layout via PE transpose; a ones column per
    head (cols 64/130 of a 132-wide slot) makes the PV matmul accumulate the
    softmax denominator in psum row 64.
  - Scores computed transposed: S^T[k, q] = K^T.T @ Q^T (K=64 -> the two
    heads' matmuls auto-pack as 64x128 row tiles and run concurrently).
  - Causal masking only touches the [128, 2, 128] diagonal square of each
    diagonal block (the triangle pattern is j-invariant); the strictly-lower
    rectangle is exp'd unmasked. Off-diagonal q-columns are never computed.
  - exp on ScalarE (key mask as per-partition bias), output bf16.
  - Softmax denominators inverted with reciprocal_approx_fast (DVE custom
    op), broadcast via DMA, normalization multiply on DVE -> attnT bf16.
  - Output projection accumulates the 4 pairs in PSUM; evacuation on DVE in
    bf16; partial outputs DMA'd to HBM in bf16.
  - Projection chunk c feeds attention group g=c (causality), so projection
    PE work overlaps the ScalarE-bound attention of earlier groups.
"""

import os

import numpy as np

DBG = bool(int(os.environ.get("MHADBG", "0")))

P = 128
B, S, D, H = 4, 2048, 1024, 16
HD = D // H          # 64
NCORES = 8
NP = 4               # head pairs per core (8 heads)
NC = S // 512        # 4 row chunks
NB = S // P          # 16 key blocks
KD = D // P          # 8 contraction chunks
FPC = NP * P         # 512 features per core
VW = 2 * (HD + 2)    # 132: per-pair V slot width (64 + ones + pad, x2 heads)

_CACHE = {}


def _build_nc():
    import concourse.mybir as mybir
    from concourse import bacc
    from concourse.tile import TileContext
    from concourse.masks import make_identity
    from contextlib import ExitStack

    f32 = mybir.dt.float32
    bf16 = mybir.dt.bfloat16
    AF = mybir.ActivationFunctionType

    nc = bacc.Bacc("TRN2", target_bir_lowering=False, debug=False,
                   num_devices=NCORES)

    xT_d = nc.dram_tensor("xT", [P, NC, KD, 512], bf16,
                          kind="ExternalInput").ap()
    wq_d = nc.dram_tensor("wq", [P, KD, FPC], bf16, kind="ExternalInput").ap()
    wk_d = nc.dram_tensor("wk", [P, KD, FPC], bf16, kind="ExternalInput").ap()
    wv_d = nc.dram_tensor("wv", [P, KD, FPC], bf16, kind="ExternalInput").ap()
    bq_d = nc.dram_tensor("bq", [P, NP], f32, kind="ExternalInput").ap()
    bk_d = nc.dram_tensor("bk", [P, NP], f32, kind="ExternalInput").ap()
    wp_d = nc.dram_tensor("wp", [P, NP, D], bf16, kind="ExternalInput").ap()
    mb_d = nc.dram_tensor("maskb", [P, NB], f32, kind="ExternalInput").ap()
    cm_d = nc.dram_tensor("cmask", [P, 2, P], f32, kind="ExternalInput").ap()
    yp_d = nc.dram_tensor("yp", [S, D], bf16, kind="ExternalOutput").ap()
    if DBG:
        dbg_qt = nc.dram_tensor("dbg_qt", [P, NC, NP, 512], bf16,
                                kind="ExternalOutput").ap()
        dbg_kt = nc.dram_tensor("dbg_kt", [P, NC, NP, 512], bf16,
                                kind="ExternalOutput").ap()
        dbg_v = nc.dram_tensor("dbg_v", [P, NC, NP, 4, VW], bf16,
                               kind="ExternalOutput").ap()
        dbg_at = nc.dram_tensor("dbg_at", [P, NC, NP, 512], bf16,
                                kind="ExternalOutput").ap()

    with TileContext(nc) as tc:
        with ExitStack() as ctx:
            consts = ctx.enter_context(tc.tile_pool(name="consts", bufs=1))
            big = ctx.enter_context(tc.tile_pool(name="big", bufs=1))
            xpool = ctx.enter_context(tc.tile_pool(name="xpool", bufs=2))
            vtpool = ctx.enter_context(tc.tile_pool(name="vtpool", bufs=2))
            ptpool = ctx.enter_context(tc.tile_pool(name="ptpool", bufs=3))
            npool = ctx.enter_context(tc.tile_pool(name="npool", bufs=2))
            pvsbpool = ctx.enter_context(
                tc.tile_pool(name="pvsbpool", bufs=18))
            ypool = ctx.enter_context(tc.tile_pool(name="ypool", bufs=3))
            pspool = ctx.enter_context(
                tc.tile_pool(name="pspool", bufs=2, space="PSUM"))
            sc2pool = ctx.enter_context(
                tc.tile_pool(name="sc2pool", bufs=2, space="PSUM"))
            pvpool = ctx.enter_context(
                tc.tile_pool(name="pvpool", bufs=2, space="PSUM"))

            # ---- constants ----
            wq_sb = consts.tile([P, KD, FPC], bf16, tag="wq")
            wk_sb = consts.tile([P, KD, FPC], bf16, tag="wk")
            wv_sb = consts.tile([P, KD, FPC], bf16, tag="wv")
            wp_sb = consts.tile([P, NP, D], bf16, tag="wp")
            nc.sync.dma_start(wq_sb[:], wq_d)
            nc.sync.dma_start(wk_sb[:], wk_d)
            nc.sync.dma_start(wv_sb[:], wv_d)
            nc.sync.dma_start(wp_sb[:], wp_d)
            bq_sb = consts.tile([P, NP], f32, tag="bq")
            bk_sb = consts.tile([P, NP], f32, tag="bk")
            nc.sync.dma_start(bq_sb[:], bq_d)
            nc.sync.dma_start(bk_sb[:], bk_d)
            mb_sb = consts.tile([P, NB], f32, tag="mb")
            nc.sync.dma_start(mb_sb[:], mb_d)
            cm_sb = consts.tile([P, 2, P], f32, tag="cm")
            nc.sync.dma_start(cm_sb[:], cm_d)
            ident = consts.tile([P, P], f32, tag="ident")
            make_identity(nc, ident[:])

            # ---- per-chunk activations (distinct tiles -> clean deps) ----
            qt_c = [big.tile([P, NP, 512], bf16, tag=f"qt{c}",
                             name=f"qt{c}") for c in range(NC)]
            kt_c = [big.tile([P, NP, 512], bf16, tag=f"kt{c}",
                             name=f"kt{c}") for c in range(NC)]
            v_c = [big.tile([P, NP, 4, VW], bf16, tag=f"v{c}",
                            name=f"v{c}") for c in range(NC)]
            at_gp = [[big.tile([P, 512], bf16, tag=f"at{g}_{p}",
                               name=f"at{g}_{p}") for p in range(NP)]
                     for g in range(NC)]
            for c in range(NC):
                nc.vector.memset(v_c[c][:, :, :, HD], 1.0)
                nc.vector.memset(v_c[c][:, :, :, HD + 2 + HD], 1.0)

            def proj_part(c, which, xt):
                w_sb = (wq_sb, wk_sb, wv_sb)[which]
                for mt in range(NP):
                    if True:
                        ps = pspool.tile([P, 512], f32, tag="ps",
                                         name=f"pj_{c}_{which}_{mt}")
                        for o in range(KD):
                            nc.tensor.matmul(
                                ps[:], lhsT=w_sb[:, o, mt * P:(mt + 1) * P],
                                rhs=xt[:, o, :],
                                start=(o == 0), stop=(o == KD - 1))
                        if which == 0:
                            nc.vector.tensor_scalar_add(
                                qt_c[c][:, mt, :], ps[:],
                                bq_sb[:, mt:mt + 1])
                        elif which == 1:
                            nc.vector.tensor_scalar_add(
                                kt_c[c][:, mt, :], ps[:],
                                bk_sb[:, mt:mt + 1])
                        else:
                            vt = vtpool.tile([P, 512], f32, tag="vt")
                            nc.vector.tensor_copy(vt[:], ps[:])
                            for t in range(4):
                                trp = pspool.tile([P, P], f32, tag="ps",
                                                  name=f"tr_{c}_{mt}_{t}")
                                nc.tensor.transpose(
                                    trp[:], vt[:, t * P:(t + 1) * P],
                                    ident[:])
                                dst = (v_c[c][:, mt, t, :]
                                       .rearrange("p (h x) -> p h x", h=2)
                                       [:, :, 0:HD])
                                src = trp[:].rearrange(
                                    "p (h x) -> p h x", h=2)
                                nc.vector.tensor_copy(dst, src)

            def attention(pair, g, pvsb_g, dn_g):
                nkb = 4 * (g + 1)
                pvs = [pvpool.tile([P, 512], f32, tag="pv",
                                   name=f"pv_{pair}_{g}_{h}")
                       for h in range(2)]
                for kb in range(nkb):
                    j = kb - 4 * g
                    q0 = P * j if j >= 0 else 0
                    sc2 = sc2pool.tile([P, 1024], f32, tag="sc2",
                                       name=f"sc2_{pair}_{g}_{kb}")
                    for h in range(2):
                        hsl = slice(HD * h, HD * (h + 1))
                        nc.tensor.matmul(
                            sc2[:, h * 512 + q0:(h + 1) * 512],
                            lhsT=kt_c[kb // 4][hsl, pair,
                                               (kb % 4) * P:(kb % 4 + 1) * P],
                            rhs=qt_c[g][hsl, pair, q0:512],
                            start=True, stop=True)
                    sc2r = sc2[:].rearrange("p (h q) -> p h q", h=2)
                    pt = ptpool.tile([P, 2, 512], bf16, tag="pt")
                    mbc = mb_sb[:, kb:kb + 1]
                    if j >= 0:
                        sq = sc2r[:, :, q0:q0 + P]
                        nc.vector.tensor_add(sq, sq, cm_sb[:])
                        nc.scalar.activation(pt[:, :, q0:512],
                                             sc2r[:, :, q0:512],
                                             AF.Exp, bias=mbc)
                    else:
                        nc.scalar.activation(pt[:], sc2r, AF.Exp, bias=mbc)
                    for h in range(2):
                        vsl = (v_c[kb // 4][:, pair, kb % 4, :]
                               .rearrange("p (hh x) -> p hh x", hh=2)
                               [:, h, 0:HD + 1])
                        nc.tensor.matmul(
                            pvs[h][0:HD + 1, q0:512],
                            lhsT=vsl,
                            rhs=pt[:, h, q0:512],
                            start=(kb == 0), stop=(kb == nkb - 1))
                for h in range(2):
                    r = 2 * pair + h
                    pvsb = pvsbpool.tile([P, 512], f32, tag="pvsb",
                                         name=f"pvsb_{pair}_{g}_{h}")
                    nc.vector.tensor_copy(pvsb[0:HD + 1, :],
                                          pvs[h][0:HD + 1, :])
                    # gather the denominator row into dn_g (partition move)
                    nc.sync.dma_start(dn_g[r:r + 1, :], pvsb[HD:HD + 1, :])
                    pvsb_g[r] = pvsb

            def normalize(g, pvsb_g, dn_g):
                rcg = npool.tile([8, 512], f32, tag="rcg")
                nc.vector.reciprocal(rcg[:], dn_g[:])
                for pair in range(NP):
                    for h in range(2):
                        sx = npool.tile([HD, 512], f32, tag="sx")
                        r = 2 * pair + h
                        nc.sync.dma_start(
                            sx[:],
                            rcg[r:r + 1, None, :].to_broadcast((1, HD, 512)))
                        if h == 0:
                            nc.vector.tensor_mul(
                                at_gp[g][pair][0:HD, :],
                                pvsb_g[r][0:HD, :], sx[:])
                        else:
                            tmp = npool.tile([HD, 512], bf16, tag="tmp")
                            nc.vector.tensor_mul(
                                tmp[:], pvsb_g[r][0:HD, :], sx[:])
                            nc.sync.dma_start(
                                at_gp[g][pair][HD:2 * HD, :], tmp[:])

            def outproj(g):
                for qi in range(4):
                    q0 = g * 512 + qi * P
                    yb = ypool.tile([P, D], bf16, tag="yb")
                    for half in range(2):
                        ps = pspool.tile([P, 512], f32, tag="ps",
                                         name=f"yps_{g}_{qi}_{half}")
                        for pair in range(NP):
                            nc.tensor.matmul(
                                ps[:],
                                lhsT=at_gp[g][pair][:, qi * P:(qi + 1) * P],
                                rhs=wp_sb[:, pair,
                                          half * 512:(half + 1) * 512],
                                start=(pair == 0), stop=(pair == NP - 1))
                        nc.vector.tensor_copy(
                            yb[:, half * 512:(half + 1) * 512], ps[:])
                    nc.sync.dma_start(yp_d[q0:q0 + P, :], yb[:])

            # chunk 0 projection (head of the pipeline)
            xt0 = xpool.tile([P, KD, 512], bf16, tag="xt", name="xt0")
            nc.sync.dma_start(xt0[:], xT_d[:, 0])
            for w in range(3):
                proj_part(0, w, xt0)
            # steady state: attention group g = c-1 overlaps projection of
            # chunk c (group g only needs chunks <= g)
            for c in range(1, NC + 1):
                g = c - 1
                if c < NC:
                    xt = xpool.tile([P, KD, 512], bf16, tag="xt",
                                    name=f"xt{c}")
                    nc.sync.dma_start(xt[:], xT_d[:, c])
                pvsb_g = {}
                dn_g = npool.tile([8, 512], f32, tag="dn", name=f"dn{g}")
                for pair in range(NP):
                    if c < NC and pair < 3:
                        proj_part(c, pair, xt)
                    attention(pair, g, pvsb_g, dn_g)
                    if pair == 0 and g >= 1:
                        outproj(g - 1)
                normalize(g, pvsb_g, dn_g)
            outproj(NC - 1)

            if DBG:
                for c in range(NC):
                    nc.sync.dma_start(dbg_qt[:, c], qt_c[c][:])
                    nc.sync.dma_start(dbg_kt[:, c], kt_c[c][:])
                    nc.sync.dma_start(dbg_v[:, c], v_c[c][:])
                    nc.sync.dma_start(dbg_at[:, c], at_g[c][:])

    nc.compile()
    return nc


def _get_nc():
    if "nc" not in _CACHE:
        _CACHE["nc"] = _build_nc()
    return _CACHE["nc"]


def make_in_maps(x, attention_mask, Wq, bq, Wk, bk, Wv, bv, Wp, bp):
    """Host-side sharding: 8 per-core input maps (core = b * 2 + hg)."""
    import ml_dtypes
    bf = ml_dtypes.bfloat16
    x = np.asarray(x, dtype=np.float32)
    scale = np.float32(1.0 / np.sqrt(HD))
    Wq = np.asarray(Wq, np.float32) * scale
    bq = np.asarray(bq, np.float32) * scale
    Wk = np.asarray(Wk, np.float32)
    bk = np.asarray(bk, np.float32)
    Wv = np.asarray(Wv, np.float32)
    Wp = np.asarray(Wp, np.float32)
    mask = np.asarray(attention_mask).astype(np.float32)

    # causal triangle for the diagonal 128x128 square (j-invariant):
    # row p (key), col q: valid iff p <= q.
    pp = np.arange(P)[:, None]
    qq = np.arange(P)[None, :]
    tri = np.where(pp <= qq, 0.0, -1e9).astype(np.float32)
    cm = np.ascontiguousarray(
        np.broadcast_to(tri[:, None, :], (P, 2, P)))

    in_maps = []
    for core in range(NCORES):
        b, hg = divmod(core, 2)
        fsl = slice(hg * FPC, (hg + 1) * FPC)
        xT = x[b].T.reshape(KD, P, NC, 512).transpose(1, 2, 0, 3)
        mb = ((mask[b] - 1.0) * np.float32(1e9)).reshape(NB, P).T
        in_maps.append({
            "xT": np.ascontiguousarray(xT.astype(bf)),
            "wq": np.ascontiguousarray(
                Wq[:, fsl].reshape(KD, P, FPC).transpose(1, 0, 2).astype(bf)),
            "wk": np.ascontiguousarray(
                Wk[:, fsl].reshape(KD, P, FPC).transpose(1, 0, 2).astype(bf)),
            "wv": np.ascontiguousarray(
                Wv[:, fsl].reshape(KD, P, FPC).transpose(1, 0, 2).astype(bf)),
            "bq": np.ascontiguousarray(bq[fsl].reshape(NP, P).T),
            "bk": np.ascontiguousarray(bk[fsl].reshape(NP, P).T),
            "wp": np.ascontiguousarray(
                Wp[fsl, :].reshape(NP, P, D).transpose(1, 0, 2).astype(bf)),
            "maskb": np.ascontiguousarray(mb),
            "cmask": cm,
        })
    return in_maps


def run(inputs, trace=False, tmpdir=None):
    """Compile (cached) + run on 8 cores. Returns (output, BassKernelResults)."""
    from concourse import bass_utils
    nc = _get_nc()
    in_maps = make_in_maps(**inputs)
    kwargs = {}
    if trace:
        kwargs = dict(trace=True, tmpdir=tmpdir)
    res = bass_utils.run_bass_kernel_spmd(
        nc, in_maps, core_ids=list(range(NCORES)), **kwargs)
    bv = np.asarray(inputs["bv"], np.float64)
    Wp = np.asarray(inputs["Wp"], np.float64)
    bp = np.asarray(inputs["bp"], np.float64)
    const = bv @ Wp + bp  # V-bias contribution (softmax rows sum to 1) + bp
    out = np.empty((B, S, D), dtype=np.float32)
    for b in range(B):
        acc = (res.results[2 * b]["yp"].astype(np.float64)
               + res.results[2 * b + 1]["yp"].astype(np.float64) + const)
        out[b] = acc.astype(np.float32)
    return out, res


def kernel(**inputs) -> np.ndarray:
    out, _ = run(inputs, trace=False)
    return out


# revision 39
# speedup vs baseline: 1.5509x; 1.2676x over previous
"""MultiHeadAttention (B=4, S=2048, D=1024, H=16, causal + key mask) on 8 trn2 cores.

Sharding: batch x head-group. Core (b, hg) owns batch b and 8 heads (4 pairs
of 2). Host sums the two half-partials per batch and adds bp + bv@Wp (the V
bias is equivalent to a constant output offset because softmax rows sum to 1).

Per-core kernel (bf16 data path, fp32 PSUM/softmax stats):
  - x^T for its batch streamed in 4 chunks of 512 rows; projections produce
    Q^T/K^T [feat=128 (pair-of-heads), rows] per pair (W slice as lhsT).
    Q/K biases folded into the PSUM->SBUF evacuation (ScalarE Identity+bias).
  - V transposed into [row, hd] layout via PE transpose; a ones column per
    head (cols 64/130 of a 132-wide slot) makes the PV matmul accumulate the
    softmax denominator in psum row 64.
  - Scores computed transposed: S^T[k, q] = K^T.T @ Q^T (K=64 -> the two
    heads' matmuls auto-pack as 64x128 row tiles and run concurrently).
  - Causal masking only touches the [128, 2, 128] diagonal square of each
    diagonal block (the triangle pattern is j-invariant); the strictly-lower
    rectangle is exp'd unmasked. Off-diagonal q-columns are never computed.
  - exp on ScalarE (key mask as per-partition bias), output bf16.
  - Softmax denominators inverted with reciprocal_approx_fast (DVE custom
    op), broadcast via DMA, normalization multiply on DVE -> attnT bf16.
  - Output projection accumulates the 4 pairs in PSUM; evacuation on DVE in
    bf16; partial outputs DMA'd to HBM in bf16.
  - Projection chunk c feeds attention group g=c (causality), so projection
    PE work overlaps the ScalarE-bound attention of earlier groups.
"""

import os

import numpy as np

DBG = bool(int(os.environ.get("MHADBG", "0")))

P = 128
B, S, D, H = 4, 2048, 1024, 16
HD = D // H          # 64
NCORES = 8
NP = 4               # head pairs per core (8 heads)
NC = S // 512        # 4 row chunks
NB = S // P          # 16 key blocks
KD = D // P          # 8 contraction chunks
FPC = NP * P         # 512 features per core
VW = 192             # V slot: [0:64]=V_h0, [64]=ones, [65:96]=0, [96:160]=V_h1

_CACHE = {}


def _build_nc():
    import concourse.mybir as mybir
    from concourse import bacc
    from concourse.tile import TileContext
    from concourse.masks import make_identity
    from contextlib import ExitStack

    f32 = mybir.dt.float32
    f32r = mybir.dt.float32r
    bf16 = mybir.dt.bfloat16
    AF = mybir.ActivationFunctionType

    nc = bacc.Bacc("TRN2", target_bir_lowering=False, debug=False,
                   num_devices=NCORES)

    xT_d = nc.dram_tensor("xT", [P, NC, KD, 512], bf16,
                          kind="ExternalInput").ap()
    wq_d = nc.dram_tensor("wq", [P, KD, FPC], bf16, kind="ExternalInput").ap()
    wk_d = nc.dram_tensor("wk", [P, KD, FPC], bf16, kind="ExternalInput").ap()
    wv_d = nc.dram_tensor("wv", [P, KD, FPC], bf16, kind="ExternalInput").ap()
    bq_d = nc.dram_tensor("bq", [P, NP], f32, kind="ExternalInput").ap()
    bk_d = nc.dram_tensor("bk", [P, NP], f32, kind="ExternalInput").ap()
    wp_d = nc.dram_tensor("wp", [P, NP, D], bf16, kind="ExternalInput").ap()
    mb_d = nc.dram_tensor("maskb", [P, NB], f32, kind="ExternalInput").ap()
    cm_d = nc.dram_tensor("cmask", [P, 2, P], f32, kind="ExternalInput").ap()
    es_d = nc.dram_tensor("esel", [8, NP, P], f32r, kind="ExternalInput").ap()
    yp_d = nc.dram_tensor("yp", [S, D], bf16, kind="ExternalOutput").ap()
    if DBG:
        dbg_qt = nc.dram_tensor("dbg_qt", [P, NC, NP, 512], bf16,
                                kind="ExternalOutput").ap()
        dbg_kt = nc.dram_tensor("dbg_kt", [P, NC, NP, 512], bf16,
                                kind="ExternalOutput").ap()
        dbg_v = nc.dram_tensor("dbg_v", [P, NC, NP, 4, VW], bf16,
                               kind="ExternalOutput").ap()
        dbg_at = nc.dram_tensor("dbg_at", [P, NC, NP, 512], bf16,
                                kind="ExternalOutput").ap()

    with TileContext(nc) as tc:
        with ExitStack() as ctx:
            consts = ctx.enter_context(tc.tile_pool(name="consts", bufs=1))
            big = ctx.enter_context(tc.tile_pool(name="big", bufs=1))
            xpool = ctx.enter_context(tc.tile_pool(name="xpool", bufs=2))
            vtpool = ctx.enter_context(tc.tile_pool(name="vtpool", bufs=2))
            ptpool = ctx.enter_context(tc.tile_pool(name="ptpool", bufs=3))
            npool = ctx.enter_context(tc.tile_pool(name="npool", bufs=2))
            pvsbpool = ctx.enter_context(
                tc.tile_pool(name="pvsbpool", bufs=18))
            ypool = ctx.enter_context(tc.tile_pool(name="ypool", bufs=3))
            pspool = ctx.enter_context(
                tc.tile_pool(name="pspool", bufs=2, space="PSUM"))
            sc2pool = ctx.enter_context(
                tc.tile_pool(name="sc2pool", bufs=2, space="PSUM"))
            pvpool = ctx.enter_context(
                tc.tile_pool(name="pvpool", bufs=2, space="PSUM"))

            # ---- constants ----
            wq_sb = consts.tile([P, KD, FPC], bf16, tag="wq")
            wk_sb = consts.tile([P, KD, FPC], bf16, tag="wk")
            wv_sb = consts.tile([P, KD, FPC], bf16, tag="wv")
            wp_sb = consts.tile([P, NP, D], bf16, tag="wp")
            nc.sync.dma_start(wq_sb[:], wq_d)
            nc.sync.dma_start(wk_sb[:], wk_d)
            nc.sync.dma_start(wv_sb[:], wv_d)
            nc.sync.dma_start(wp_sb[:], wp_d)
            bq_sb = consts.tile([P, NP], f32, tag="bq")
            bk_sb = consts.tile([P, NP], f32, tag="bk")
            nc.sync.dma_start(bq_sb[:], bq_d)
            nc.sync.dma_start(bk_sb[:], bk_d)
            mb_sb = consts.tile([P, NB], f32, tag="mb")
            nc.sync.dma_start(mb_sb[:], mb_d)
            cm_sb = consts.tile([P, 2, P], f32, tag="cm")
            nc.sync.dma_start(cm_sb[:], cm_d)
            ident = consts.tile([P, P], f32, tag="ident")
            make_identity(nc, ident[:])

            # ---- per-chunk activations (distinct tiles -> clean deps) ----
            qt_c = [big.tile([P, NP, 512], bf16, tag=f"qt{c}",
                             name=f"qt{c}") for c in range(NC)]
            kt_c = [big.tile([P, NP, 512], bf16, tag=f"kt{c}",
                             name=f"kt{c}") for c in range(NC)]
            v_c = [big.tile([P, NP, 4, VW], bf16, tag=f"v{c}",
                            name=f"v{c}") for c in range(NC)]
            at_gp = [[big.tile([P, 512], bf16, tag=f"at{g}_{p}",
                               name=f"at{g}_{p}") for p in range(NP)]
                     for g in range(NC)]
            for c in range(NC):
                nc.vector.memset(v_c[c][:, :, :, HD], 1.0)
                nc.vector.memset(v_c[c][:, :, :, HD + 1:96], 0.0)
            es_sb = consts.tile([8, NP, P], f32r, tag="esel")
            nc.sync.dma_start(es_sb[:], es_d)

            def proj_part(c, which, xt):
                w_sb = (wq_sb, wk_sb, wv_sb)[which]
                for mt in range(NP):
                    if True:
                        ps = pspool.tile([P, 512], f32, tag="ps",
                                         name=f"pj_{c}_{which}_{mt}")
                        for o in range(KD):
                            nc.tensor.matmul(
                                ps[:], lhsT=w_sb[:, o, mt * P:(mt + 1) * P],
                                rhs=xt[:, o, :],
                                start=(o == 0), stop=(o == KD - 1))
                        if which == 0:
                            nc.vector.tensor_scalar_add(
                                qt_c[c][:, mt, :], ps[:],
                                bq_sb[:, mt:mt + 1])
                        elif which == 1:
                            nc.vector.tensor_scalar_add(
                                kt_c[c][:, mt, :], ps[:],
                                bk_sb[:, mt:mt + 1])
                        else:
                            vt = vtpool.tile([P, 512], f32, tag="vt")
                            nc.vector.tensor_copy(vt[:], ps[:])
                            for t in range(4):
                                trp = pspool.tile([P, P], f32, tag="ps",
                                                  name=f"tr_{c}_{mt}_{t}")
                                nc.tensor.transpose(
                                    trp[:], vt[:, t * P:(t + 1) * P],
                                    ident[:])
                                dst = (v_c[c][:, mt, t, :]
                                       .rearrange("p (h x) -> p h x", h=2)
                                       [:, :, 0:HD])
                                src = trp[:].rearrange(
                                    "p (h x) -> p h x", h=2)
                                nc.vector.tensor_copy(dst, src)

            def attention(pair, g, pvsb_g, dn_g):
                nkb = 4 * (g + 1)
                pvs = [pvpool.tile([P, 512], f32, tag="pv",
                                   name=f"pv_{pair}_{g}_{h}")
                       for h in range(2)]
                for kb in range(nkb):
                    j = kb - 4 * g
                    q0 = P * j if j >= 0 else 0
                    sc2 = sc2pool.tile([P, 1024], f32, tag="sc2",
                                       name=f"sc2_{pair}_{g}_{kb}")
                    for h in range(2):
                        hsl = slice(HD * h, HD * (h + 1))
                        nc.tensor.matmul(
                            sc2[:, h * 512 + q0:(h + 1) * 512],
                            lhsT=kt_c[kb // 4][hsl, pair,
                                               (kb % 4) * P:(kb % 4 + 1) * P],
                            rhs=qt_c[g][hsl, pair, q0:512],
                            start=True, stop=True)
                    sc2r = sc2[:].rearrange("p (h q) -> p h q", h=2)
                    pt = ptpool.tile([P, 2, 512], bf16, tag="pt")
                    mbc = mb_sb[:, kb:kb + 1]
                    if j >= 0:
                        sq = sc2r[:, :, q0:q0 + P]
                        nc.vector.tensor_add(sq, sq, cm_sb[:])
                        nc.scalar.activation(pt[:, :, q0:512],
                                             sc2r[:, :, q0:512],
                                             AF.Exp, bias=mbc)
                    else:
                        nc.scalar.activation(pt[:], sc2r, AF.Exp, bias=mbc)
                    vb = v_c[kb // 4][:, pair, kb % 4, :]
                    # h0: lhsT cols [0:65] -> rows 0-63 data, row 64 denom.
                    # h1: lhsT cols [32:160] -> row 32 denom (the shared ones
                    #     column), rows 64-127 data (partition-aligned with
                    #     at_gp's head-1 half: no cross-partition move).
                    nc.tensor.matmul(
                        pvs[0][0:HD + 1, q0:512], lhsT=vb[:, 0:HD + 1],
                        rhs=pt[:, 0, q0:512],
                        start=(kb == 0), stop=(kb == nkb - 1))
                    nc.tensor.matmul(
                        pvs[1][:, q0:512], lhsT=vb[:, 32:160],
                        rhs=pt[:, 1, q0:512],
                        start=(kb == 0), stop=(kb == nkb - 1))
                for h in range(2):
                    r = 2 * pair + h
                    pvsb = pvsbpool.tile([P, 512], f32, tag="pvsb",
                                         name=f"pvsb_{pair}_{g}_{h}")
                    dr = HD if h == 0 else 32          # denominator row
                    if h == 0:
                        nc.vector.tensor_copy(pvsb[0:HD + 1, :],
                                              pvs[h][0:HD + 1, :])
                    else:
                        nc.vector.tensor_copy(pvsb[:], pvs[h][:])
                    # gather the denominator row into dn_g (partition move)
                    nc.sync.dma_start(dn_g[r:r + 1, :], pvsb[dr:dr + 1, :])
                    pvsb_g[r] = pvsb

            def normalize(g, pvsb_g, dn_g):
                rcg = npool.tile([8, 512], f32r, tag="rcg")
                with nc.allow_low_precision(reason="f32r broadcast matmul"):
                    nc.vector.reciprocal(rcg[:], dn_g[:])
                for pair in range(NP):
                    # broadcast 1/denom to the pair's 128 partitions with a
                    # K=8 selection matmul (avoids slow single-partition-
                    # source broadcast DMAs entirely)
                    sxp = pspool.tile([P, 512], f32, tag="ps",
                                      name=f"sx_{g}_{pair}")
                    nc.tensor.matmul(
                        sxp[:], lhsT=es_sb[:, pair, :],
                        rhs=rcg[:], start=True, stop=True)
                    nc.vector.tensor_mul(
                        at_gp[g][pair][0:HD, :],
                        pvsb_g[2 * pair][0:HD, :], sxp[0:HD, :])
                    nc.vector.tensor_mul(
                        at_gp[g][pair][HD:2 * HD, :],
                        pvsb_g[2 * pair + 1][HD:2 * HD, :],
                        sxp[HD:2 * HD, :])

            def outproj(g):
                for qi in range(4):
                    q0 = g * 512 + qi * P
                    yb = ypool.tile([P, D], bf16, tag="yb")
                    for half in range(2):
                        ps = pspool.tile([P, 512], f32, tag="ps",
                                         name=f"yps_{g}_{qi}_{half}")
                        for pair in range(NP):
                            nc.tensor.matmul(
                                ps[:],
                                lhsT=at_gp[g][pair][:, qi * P:(qi + 1) * P],
                                rhs=wp_sb[:, pair,
                                          half * 512:(half + 1) * 512],
                                start=(pair == 0), stop=(pair == NP - 1))
                        nc.vector.tensor_copy(
                            yb[:, half * 512:(half + 1) * 512], ps[:])
                    nc.sync.dma_start(yp_d[q0:q0 + P, :], yb[:])

            # chunk 0 projection (head of the pipeline)
            xt0 = xpool.tile([P, KD, 512], bf16, tag="xt", name="xt0")
            nc.sync.dma_start(xt0[:], xT_d[:, 0])
            for w in range(3):
                proj_part(0, w, xt0)
            # steady state: attention group g = c-1 overlaps projection of
            # chunk c (group g only needs chunks <= g)
            for c in range(1, NC + 1):
                g = c - 1
                if c < NC:
                    xt = xpool.tile([P, KD, 512], bf16, tag="xt",
                                    name=f"xt{c}")
                    nc.sync.dma_start(xt[:], xT_d[:, c])
                pvsb_g = {}
                dn_g = npool.tile([8, 512], f32, tag="dn", name=f"dn{g}")
                for pair in range(NP):
                    if c < NC and pair < 3:
                        proj_part(c, pair, xt)
                    attention(pair, g, pvsb_g, dn_g)
                    if pair == 0 and g >= 1:
                        outproj(g - 1)
                normalize(g, pvsb_g, dn_g)
            outproj(NC - 1)

            if DBG:
                for c in range(NC):
                    nc.sync.dma_start(dbg_qt[:, c], qt_c[c][:])
                    nc.sync.dma_start(dbg_kt[:, c], kt_c[c][:])
                    nc.sync.dma_start(dbg_v[:, c], v_c[c][:])
                    nc.sync.dma_start(dbg_at[:, c], at_g[c][:])

    nc.compile()
    return nc


def _get_nc():
    if "nc" not in _CACHE:
        _CACHE["nc"] = _build_nc()
    return _CACHE["nc"]


def make_in_maps(x, attention_mask, Wq, bq, Wk, bk, Wv, bv, Wp, bp):
    """Host-side sharding: 8 per-core input maps (core = b * 2 + hg)."""
    import ml_dtypes
    bf = ml_dtypes.bfloat16
    x = np.asarray(x, dtype=np.float32)
    scale = np.float32(1.0 / np.sqrt(HD))
    Wq = np.asarray(Wq, np.float32) * scale
    bq = np.asarray(bq, np.float32) * scale
    Wk = np.asarray(Wk, np.float32)
    bk = np.asarray(bk, np.float32)
    Wv = np.asarray(Wv, np.float32)
    Wp = np.asarray(Wp, np.float32)
    mask = np.asarray(attention_mask).astype(np.float32)

    # causal triangle for the diagonal 128x128 square (j-invariant):
    # row p (key), col q: valid iff p <= q.
    pp = np.arange(P)[:, None]
    qq = np.arange(P)[None, :]
    tri = np.where(pp <= qq, 0.0, -1e9).astype(np.float32)
    cm = np.ascontiguousarray(
        np.broadcast_to(tri[:, None, :], (P, 2, P)))

    # selection matrix for the reciprocal-broadcast matmul:
    # esel[r, pair, m] = 1 iff r == 2*pair + m//64
    es = np.zeros((8, NP, P), dtype=np.float32)
    for pr in range(NP):
        es[2 * pr, pr, 0:HD] = 1.0
        es[2 * pr + 1, pr, HD:P] = 1.0

    in_maps = []
    for core in range(NCORES):
        b, hg = divmod(core, 2)
        fsl = slice(hg * FPC, (hg + 1) * FPC)
        xT = x[b].T.reshape(KD, P, NC, 512).transpose(1, 2, 0, 3)
        mb = ((mask[b] - 1.0) * np.float32(1e9)).reshape(NB, P).T
        in_maps.append({
            "xT": np.ascontiguousarray(xT.astype(bf)),
            "wq": np.ascontiguousarray(
                Wq[:, fsl].reshape(KD, P, FPC).transpose(1, 0, 2).astype(bf)),
            "wk": np.ascontiguousarray(
                Wk[:, fsl].reshape(KD, P, FPC).transpose(1, 0, 2).astype(bf)),
            "wv": np.ascontiguousarray(
                Wv[:, fsl].reshape(KD, P, FPC).transpose(1, 0, 2).astype(bf)),
            "bq": np.ascontiguousarray(bq[fsl].reshape(NP, P).T),
            "bk": np.ascontiguousarray(bk[fsl].reshape(NP, P).T),
            "wp": np.ascontiguousarray(
                Wp[fsl, :].reshape(NP, P, D).transpose(1, 0, 2).astype(bf)),
            "maskb": np.ascontiguousarray(mb),
            "cmask": cm,
            "esel": es,
        })
    return in_maps


def run(inputs, trace=False, tmpdir=None):
    """Compile (cached) + run on 8 cores. Returns (output, BassKernelResults)."""
    from concourse import bass_utils
    nc = _get_nc()
    in_maps = make_in_maps(**inputs)
    kwargs = {}
    if trace:
        kwargs = dict(trace=True, tmpdir=tmpdir)
    res = bass_utils.run_bass_kernel_spmd(
        nc, in_maps, core_ids=list(range(NCORES)), **kwargs)
    bv = np.asarray(inputs["bv"], np.float64)
    Wp = np.asarray(inputs["Wp"], np.float64)
    bp = np.asarray(inputs["bp"], np.float64)
    const = bv @ Wp + bp  # V-bias contribution (softmax rows sum to 1) + bp
    out = np.empty((B, S, D), dtype=np.float32)
    for b in range(B):
        acc = (res.results[2 * b]["yp"].astype(np.float64)
               + res.results[2 * b + 1]["yp"].astype(np.float64) + const)
        out[b] = acc.astype(np.float32)
    return out, res


def kernel(**inputs) -> np.ndarray:
    out, _ = run(inputs, trace=False)
    return out


# revision 40
# speedup vs baseline: 1.6052x; 1.0350x over previous
"""MultiHeadAttention (B=4, S=2048, D=1024, H=16, causal + key mask) on 8 trn2 cores.

Sharding: batch x head-group. Core (b, hg) owns batch b and 8 heads (4 pairs
of 2). Host sums the two half-partials per batch and adds bp + bv@Wp (the V
bias is equivalent to a constant output offset because softmax rows sum to 1).

Per-core kernel (bf16 data path, fp32 PSUM/softmax stats):
  - x^T for its batch streamed in 4 chunks of 512 rows; projections produce
    Q^T/K^T [feat=128 (pair-of-heads), rows] per pair (W slice as lhsT).
    Q/K biases folded into the PSUM->SBUF evacuation (ScalarE Identity+bias).
  - V transposed into [row, hd] layout via PE transpose; a ones column per
    head (cols 64/130 of a 132-wide slot) makes the PV matmul accumulate the
    softmax denominator in psum row 64.
  - Scores computed transposed: S^T[k, q] = K^T.T @ Q^T (K=64 -> the two
    heads' matmuls auto-pack as 64x128 row tiles and run concurrently).
  - Causal masking only touches the [128, 2, 128] diagonal square of each
    diagonal block (the triangle pattern is j-invariant); the strictly-lower
    rectangle is exp'd unmasked. Off-diagonal q-columns are never computed.
  - exp on ScalarE (key mask as per-partition bias), output bf16.
  - Softmax denominators inverted with reciprocal_approx_fast (DVE custom
    op), broadcast via DMA, normalization multiply on DVE -> attnT bf16.
  - Output projection accumulates the 4 pairs in PSUM; evacuation on DVE in
    bf16; partial outputs DMA'd to HBM in bf16.
  - Projection chunk c feeds attention group g=c (causality), so projection
    PE work overlaps the ScalarE-bound attention of earlier groups.
"""

import os

import numpy as np

DBG = bool(int(os.environ.get("MHADBG", "0")))

P = 128
B, S, D, H = 4, 2048, 1024, 16
HD = D // H          # 64
NCORES = 8
NP = 4               # head pairs per core (8 heads)
NC = S // 512        # 4 row chunks
NB = S // P          # 16 key blocks
KD = D // P          # 8 contraction chunks
FPC = NP * P         # 512 features per core
VW = 192             # V slot: [0:64]=V_h0, [64]=ones, [65:96]=0, [96:160]=V_h1

_CACHE = {}


def _build_nc():
    import concourse.mybir as mybir
    from concourse import bacc
    from concourse.tile import TileContext
    from concourse.masks import make_identity
    from contextlib import ExitStack

    f32 = mybir.dt.float32
    f32r = mybir.dt.float32r
    bf16 = mybir.dt.bfloat16
    AF = mybir.ActivationFunctionType

    nc = bacc.Bacc("TRN2", target_bir_lowering=False, debug=False,
                   num_devices=NCORES)

    xT_d = nc.dram_tensor("xT", [P, NC, KD, 512], bf16,
                          kind="ExternalInput").ap()
    wq_d = nc.dram_tensor("wq", [P, KD, FPC], bf16, kind="ExternalInput").ap()
    wk_d = nc.dram_tensor("wk", [P, KD, FPC], bf16, kind="ExternalInput").ap()
    wv_d = nc.dram_tensor("wv", [P, KD, FPC], bf16, kind="ExternalInput").ap()
    bq_d = nc.dram_tensor("bq", [P, NP], f32, kind="ExternalInput").ap()
    bk_d = nc.dram_tensor("bk", [P, NP], f32, kind="ExternalInput").ap()
    wp_d = nc.dram_tensor("wp", [P, NP, D], bf16, kind="ExternalInput").ap()
    mb_d = nc.dram_tensor("maskb", [P, NB], f32, kind="ExternalInput").ap()
    cm_d = nc.dram_tensor("cmask", [P, 2, P], bf16,
                          kind="ExternalInput").ap()
    es_d = nc.dram_tensor("esel", [8, NP, P], f32r, kind="ExternalInput").ap()
    yp_d = nc.dram_tensor("yp", [S, D], bf16, kind="ExternalOutput").ap()
    if DBG:
        dbg_qt = nc.dram_tensor("dbg_qt", [P, NC, NP, 512], bf16,
                                kind="ExternalOutput").ap()
        dbg_kt = nc.dram_tensor("dbg_kt", [P, NC, NP, 512], bf16,
                                kind="ExternalOutput").ap()
        dbg_v = nc.dram_tensor("dbg_v", [P, NC, NP, 4, VW], bf16,
                               kind="ExternalOutput").ap()
        dbg_at = nc.dram_tensor("dbg_at", [P, NC, NP, 512], bf16,
                                kind="ExternalOutput").ap()

    with TileContext(nc) as tc:
        with ExitStack() as ctx:
            consts = ctx.enter_context(tc.tile_pool(name="consts", bufs=1))
            big = ctx.enter_context(tc.tile_pool(name="big", bufs=1))
            xpool = ctx.enter_context(tc.tile_pool(name="xpool", bufs=2))
            vtpool = ctx.enter_context(tc.tile_pool(name="vtpool", bufs=2))
            ptpool = ctx.enter_context(tc.tile_pool(name="ptpool", bufs=4))
            npool = ctx.enter_context(tc.tile_pool(name="npool", bufs=2))
            pvsbpool = ctx.enter_context(
                tc.tile_pool(name="pvsbpool", bufs=18))
            ypool = ctx.enter_context(tc.tile_pool(name="ypool", bufs=3))
            pspool = ctx.enter_context(
                tc.tile_pool(name="pspool", bufs=2, space="PSUM"))
            sc2pool = ctx.enter_context(
                tc.tile_pool(name="sc2pool", bufs=2, space="PSUM"))
            pvpool = ctx.enter_context(
                tc.tile_pool(name="pvpool", bufs=2, space="PSUM"))

            # ---- constants ----
            wq_sb = consts.tile([P, KD, FPC], bf16, tag="wq")
            wk_sb = consts.tile([P, KD, FPC], bf16, tag="wk")
            wv_sb = consts.tile([P, KD, FPC], bf16, tag="wv")
            wp_sb = consts.tile([P, NP, D], bf16, tag="wp")
            nc.sync.dma_start(wq_sb[:], wq_d)
            nc.sync.dma_start(wk_sb[:], wk_d)
            nc.sync.dma_start(wv_sb[:], wv_d)
            bq_sb = consts.tile([P, NP], f32, tag="bq")
            bk_sb = consts.tile([P, NP], f32, tag="bk")
            nc.sync.dma_start(bq_sb[:], bq_d)
            nc.sync.dma_start(bk_sb[:], bk_d)
            mb_sb = consts.tile([P, NB], f32, tag="mb")
            nc.sync.dma_start(mb_sb[:], mb_d)
            cm_sb = consts.tile([P, 2, P], bf16, tag="cm")
            nc.sync.dma_start(cm_sb[:], cm_d)
            ident = consts.tile([P, P], f32, tag="ident")
            make_identity(nc, ident[:])

            # ---- per-chunk activations (distinct tiles -> clean deps) ----
            qt_c = [big.tile([P, NP, 512], bf16, tag=f"qt{c}",
                             name=f"qt{c}") for c in range(NC)]
            kt_c = [big.tile([P, NP, 512], bf16, tag=f"kt{c}",
                             name=f"kt{c}") for c in range(NC)]
            v_c = [big.tile([P, NP, 4, VW], bf16, tag=f"v{c}",
                            name=f"v{c}") for c in range(NC)]
            at_gp = [[big.tile([P, 512], bf16, tag=f"at{g}_{p}",
                               name=f"at{g}_{p}") for p in range(NP)]
                     for g in range(NC)]
            for c in range(NC):
                nc.vector.memset(v_c[c][:, :, :, HD], 1.0)
                nc.vector.memset(v_c[c][:, :, :, HD + 1:96], 0.0)
            es_sb = consts.tile([8, NP, P], f32r, tag="esel")
            nc.sync.dma_start(es_sb[:], es_d)
            nc.sync.dma_start(wp_sb[:], wp_d)

            def proj_part(c, which, xt):
                w_sb = (wq_sb, wk_sb, wv_sb)[which]
                for mt in range(NP):
                    if True:
                        ps = pspool.tile([P, 512], f32, tag="ps",
                                         name=f"pj_{c}_{which}_{mt}")
                        for o in range(KD):
                            nc.tensor.matmul(
                                ps[:], lhsT=w_sb[:, o, mt * P:(mt + 1) * P],
                                rhs=xt[:, o, :],
                                start=(o == 0), stop=(o == KD - 1))
                        if which == 0:
                            nc.vector.tensor_scalar_add(
                                qt_c[c][:, mt, :], ps[:],
                                bq_sb[:, mt:mt + 1])
                        elif which == 1:
                            nc.vector.tensor_scalar_add(
                                kt_c[c][:, mt, :], ps[:],
                                bk_sb[:, mt:mt + 1])
                        else:
                            vt = vtpool.tile([P, 512], f32, tag="vt")
                            nc.vector.tensor_copy(vt[:], ps[:])
                            for t in range(4):
                                trp = pspool.tile([P, P], f32, tag="ps",
                                                  name=f"tr_{c}_{mt}_{t}")
                                nc.tensor.transpose(
                                    trp[:], vt[:, t * P:(t + 1) * P],
                                    ident[:])
                                dst = (v_c[c][:, mt, t, :]
                                       .rearrange("p (h x) -> p h x", h=2)
                                       [:, :, 0:HD])
                                src = trp[:].rearrange(
                                    "p (h x) -> p h x", h=2)
                                nc.vector.tensor_copy(dst, src)

            def attention(pair, g, pvsb_g, dn_g):
                nkb = 4 * (g + 1)
                pvs = [pvpool.tile([P, 512], f32, tag="pv",
                                   name=f"pv_{pair}_{g}_{h}")
                       for h in range(2)]
                for kb in range(nkb):
                    j = kb - 4 * g
                    q0 = P * j if j >= 0 else 0
                    sc2 = sc2pool.tile([P, 1024], f32, tag="sc2",
                                       name=f"sc2_{pair}_{g}_{kb}")
                    for h in range(2):
                        hsl = slice(HD * h, HD * (h + 1))
                        nc.tensor.matmul(
                            sc2[:, h * 512 + q0:(h + 1) * 512],
                            lhsT=kt_c[kb // 4][hsl, pair,
                                               (kb % 4) * P:(kb % 4 + 1) * P],
                            rhs=qt_c[g][hsl, pair, q0:512],
                            start=True, stop=True)
                    sc2r = sc2[:].rearrange("p (h q) -> p h q", h=2)
                    pt = ptpool.tile([P, 2, 512], bf16, tag="pt")
                    mbc = mb_sb[:, kb:kb + 1]
                    if j >= 0:
                        # exp first (keeps ScalarE off the DVE critical
                        # path), then zero the causal triangle of the
                        # diagonal square multiplicatively on bf16
                        nc.scalar.activation(pt[:, :, q0:512],
                                             sc2r[:, :, q0:512],
                                             AF.Exp, bias=mbc)
                        sq = pt[:, :, q0:q0 + P]
                        nc.vector.tensor_mul(sq, sq, cm_sb[:])
                    else:
                        nc.scalar.activation(pt[:], sc2r, AF.Exp, bias=mbc)
                    vb = v_c[kb // 4][:, pair, kb % 4, :]
                    # h0: lhsT cols [0:65] -> rows 0-63 data, row 64 denom.
                    # h1: lhsT cols [32:160] -> row 32 denom (the shared ones
                    #     column), rows 64-127 data (partition-aligned with
                    #     at_gp's head-1 half: no cross-partition move).
                    nc.tensor.matmul(
                        pvs[0][0:HD + 1, q0:512], lhsT=vb[:, 0:HD + 1],
                        rhs=pt[:, 0, q0:512],
                        start=(kb == 0), stop=(kb == nkb - 1))
                    nc.tensor.matmul(
                        pvs[1][:, q0:512], lhsT=vb[:, 32:160],
                        rhs=pt[:, 1, q0:512],
                        start=(kb == 0), stop=(kb == nkb - 1))
                for h in range(2):
                    r = 2 * pair + h
                    pvsb = pvsbpool.tile([P, 512], f32, tag="pvsb",
                                         name=f"pvsb_{pair}_{g}_{h}")
                    dr = HD if h == 0 else 32          # denominator row
                    if h == 0:
                        nc.vector.tensor_copy(pvsb[0:HD + 1, :],
                                              pvs[h][0:HD + 1, :])
                    else:
                        nc.vector.tensor_copy(pvsb[:], pvs[h][:])
                    # gather the denominator row into dn_g (partition move)
                    nc.sync.dma_start(dn_g[r:r + 1, :], pvsb[dr:dr + 1, :])
                    pvsb_g[r] = pvsb

            def normalize(g, pvsb_g, dn_g):
                rcg = npool.tile([8, 512], f32r, tag="rcg")
                with nc.allow_low_precision(reason="f32r broadcast matmul"):
                    nc.vector.reciprocal(rcg[:], dn_g[:])
                for pair in range(NP):
                    # broadcast 1/denom to the pair's 128 partitions with a
                    # K=8 selection matmul (avoids slow single-partition-
                    # source broadcast DMAs entirely)
                    sxp = pspool.tile([P, 512], f32, tag="ps",
                                      name=f"sx_{g}_{pair}")
                    nc.tensor.matmul(
                        sxp[:], lhsT=es_sb[:, pair, :],
                        rhs=rcg[:], start=True, stop=True)
                    nc.vector.tensor_mul(
                        at_gp[g][pair][0:HD, :],
                        pvsb_g[2 * pair][0:HD, :], sxp[0:HD, :])
                    nc.vector.tensor_mul(
                        at_gp[g][pair][HD:2 * HD, :],
                        pvsb_g[2 * pair + 1][HD:2 * HD, :],
                        sxp[HD:2 * HD, :])

            def outproj(g):
                for qi in range(4):
                    q0 = g * 512 + qi * P
                    yb = ypool.tile([P, D], bf16, tag="yb")
                    for half in range(2):
                        ps = pspool.tile([P, 512], f32, tag="ps",
                                         name=f"yps_{g}_{qi}_{half}")
                        for pair in range(NP):
                            nc.tensor.matmul(
                                ps[:],
                                lhsT=at_gp[g][pair][:, qi * P:(qi + 1) * P],
                                rhs=wp_sb[:, pair,
                                          half * 512:(half + 1) * 512],
                                start=(pair == 0), stop=(pair == NP - 1))
                        nc.vector.tensor_copy(
                            yb[:, half * 512:(half + 1) * 512], ps[:])
                    nc.sync.dma_start(yp_d[q0:q0 + P, :], yb[:])

            # chunk 0 projection (head of the pipeline)
            xt0 = xpool.tile([P, KD, 512], bf16, tag="xt", name="xt0")
            nc.sync.dma_start(xt0[:], xT_d[:, 0])
            for w in (1, 2, 0):
                proj_part(0, w, xt0)
            # steady state: attention group g = c-1 overlaps projection of
            # chunk c (group g only needs chunks <= g)
            for c in range(1, NC + 1):
                g = c - 1
                if c < NC:
                    xt = xpool.tile([P, KD, 512], bf16, tag="xt",
                                    name=f"xt{c}")
                    nc.sync.dma_start(xt[:], xT_d[:, c])
                pvsb_g = {}
                dn_g = npool.tile([8, 512], f32, tag="dn", name=f"dn{g}")
                for pair in range(NP):
                    if c < NC and pair < 3:
                        proj_part(c, pair, xt)
                    attention(pair, g, pvsb_g, dn_g)
                    if pair == 0 and g >= 1:
                        outproj(g - 1)
                normalize(g, pvsb_g, dn_g)
            outproj(NC - 1)

            if DBG:
                for c in range(NC):
                    nc.sync.dma_start(dbg_qt[:, c], qt_c[c][:])
                    nc.sync.dma_start(dbg_kt[:, c], kt_c[c][:])
                    nc.sync.dma_start(dbg_v[:, c], v_c[c][:])
                    nc.sync.dma_start(dbg_at[:, c], at_g[c][:])

    nc.compile()
    return nc


def _get_nc():
    if "nc" not in _CACHE:
        _CACHE["nc"] = _build_nc()
    return _CACHE["nc"]


def make_in_maps(x, attention_mask, Wq, bq, Wk, bk, Wv, bv, Wp, bp):
    """Host-side sharding: 8 per-core input maps (core = b * 2 + hg)."""
    import ml_dtypes
    bf = ml_dtypes.bfloat16
    x = np.asarray(x, dtype=np.float32)
    scale = np.float32(1.0 / np.sqrt(HD))
    Wq = np.asarray(Wq, np.float32) * scale
    bq = np.asarray(bq, np.float32) * scale
    Wk = np.asarray(Wk, np.float32)
    bk = np.asarray(bk, np.float32)
    Wv = np.asarray(Wv, np.float32)
    Wp = np.asarray(Wp, np.float32)
    mask = np.asarray(attention_mask).astype(np.float32)

    # causal triangle for the diagonal 128x128 square (j-invariant):
    # row p (key), col q: valid iff p <= q.
    pp = np.arange(P)[:, None]
    qq = np.arange(P)[None, :]
    tri = np.where(pp <= qq, 1.0, 0.0).astype(np.float32)
    cm = np.ascontiguousarray(
        np.broadcast_to(tri[:, None, :], (P, 2, P)).astype(ml_dtypes.bfloat16))

    # selection matrix for the reciprocal-broadcast matmul:
    # esel[r, pair, m] = 1 iff r == 2*pair + m//64
    es = np.zeros((8, NP, P), dtype=np.float32)
    for pr in range(NP):
        es[2 * pr, pr, 0:HD] = 1.0
        es[2 * pr + 1, pr, HD:P] = 1.0

    in_maps = []
    for core in range(NCORES):
        b, hg = divmod(core, 2)
        fsl = slice(hg * FPC, (hg + 1) * FPC)
        xT = x[b].T.reshape(KD, P, NC, 512).transpose(1, 2, 0, 3)
        mb = ((mask[b] - 1.0) * np.float32(1e9)).reshape(NB, P).T
        in_maps.append({
            "xT": np.ascontiguousarray(xT.astype(bf)),
            "wq": np.ascontiguousarray(
                Wq[:, fsl].reshape(KD, P, FPC).transpose(1, 0, 2).astype(bf)),
            "wk": np.ascontiguousarray(
                Wk[:, fsl].reshape(KD, P, FPC).transpose(1, 0, 2).astype(bf)),
            "wv": np.ascontiguousarray(
                Wv[:, fsl].reshape(KD, P, FPC).transpose(1, 0, 2).astype(bf)),
            "bq": np.ascontiguousarray(bq[fsl].reshape(NP, P).T),
            "bk": np.ascontiguousarray(bk[fsl].reshape(NP, P).T),
            "wp": np.ascontiguousarray(
                Wp[fsl, :].reshape(NP, P, D).transpose(1, 0, 2).astype(bf)),
            "maskb": np.ascontiguousarray(mb),
            "cmask": cm,
            "esel": es,
        })
    return in_maps


def run(inputs, trace=False, tmpdir=None):
    """Compile (cached) + run on 8 cores. Returns (output, BassKernelResults)."""
    from concourse import bass_utils
    nc = _get_nc()
    in_maps = make_in_maps(**inputs)
    kwargs = {}
    if trace:
        kwargs = dict(trace=True, tmpdir=tmpdir)
    res = bass_utils.run_bass_kernel_spmd(
        nc, in_maps, core_ids=list(range(NCORES)), **kwargs)
    bv = np.asarray(inputs["bv"], np.float64)
    Wp = np.asarray(inputs["Wp"], np.float64)
    bp = np.asarray(inputs["bp"], np.float64)
    const = bv @ Wp + bp  # V-bias contribution (softmax rows sum to 1) + bp
    out = np.empty((B, S, D), dtype=np.float32)
    for b in range(B):
        acc = (res.results[2 * b]["yp"].astype(np.float64)
               + res.results[2 * b + 1]["yp"].astype(np.float64) + const)
        out[b] = acc.astype(np.float32)
    return out, res


def kernel(**inputs) -> np.ndarray:
    out, _ = run(inputs, trace=False)
    return out
